# revision 1
# baseline (speedup 1.0000x reference)
"""Trainium2 Bass kernel for ragged-sequence attention (G2/f16/split-DMA).

Per batch b:
    tq     = tanh(query[b] @ W + bias)                      [CA, H]
    scores = key[b] @ tq.T                                  [S, CA]
    alpha  = exp(scores) * (s < seq_len[b])                 [S, CA]
    out[b] = (alpha.T @ value[b]) / alpha.sum(axis=0)[:,None]

Strategy (HBM-bandwidth bound; everything serves DMA bytes):
  - Raggedness: independent 128-row sub-chunks of each valid prefix;
    numerator/denominator are additive over s, each sub yields a partial
    [CA, 768+1] (col 768 = denominator via a ones-column in the value tile).
  - Subs are spread round-robin over 8 cores, packed 2 per "group"; one
    group = two DMAs (~0.5MB keyT/tq/mask half on the SP HWDGE ring, ~0.4MB
    value half on the ACT HWDGE ring) for parallel descriptor streams.
    Identical NEFF on all cores (SPMD); dummy subs have zero tq/mask.
    Host does the tiny group-by-batch reduction and division.
  - Streams in fp16 (better mantissa than bf16 at equal bytes; inputs are
    O(1)-ranged so fp16's range is ample). BASS_ATTN_DT=bf16 / f32r
    switch the stream dtype. exp and psum accumulation stay fp32;
    partial outputs return as fp16.
  - key is pre-transposed on the host into [128, 6, 128] h-major tiles
    (4-byte dtypes have no DMA-transpose path, and the host does it for
    free); value tiles are [128, 772] s-major with ones at col 768.
"""

import os
import sys

import numpy as np

for _p in ("/opt/trn_rl_repo", "/root/.axon_site/_ro/trn_rl_repo"):
    if os.path.isdir(_p) and _p not in sys.path:
        sys.path.append(_p)

N_CORES = 8
SUB = 128        # rows per work item (= matmul contraction dim)
G = 3            # sub-chunks per group (one DMA / processing slot)
H = 768
HSUB = H // 128  # 6
CA = 32
VW = 772         # value tile width: 768 value cols + ones col @768 + pad
NQ = VW // 4     # 193: value matmul runs as 4 PE col-tiles -> one psum bank

TQ_W = HSUB * CA              # 192 per sub
TQ_OFF = 0
MK_OFF = TQ_OFF + G * TQ_W    # 384
MK_W = G                      # 2
ID_OFF = MK_OFF + MK_W        # 386
ID_W = CA                     # 32
KT_OFF = ID_OFF + ID_W        # 418
KT_W = HSUB * SUB             # 768 per sub
VL_OFF = KT_OFF + G * KT_W    # 1954
COMB_W = VL_OFF + G * VW      # 3498

DT = os.environ.get("BASS_ATTN_DT", "f16")

_module_cache = {}
_last_in_maps = None


def _np_dt():
    if DT == "bf16":
        import ml_dtypes

        return ml_dtypes.bfloat16
    if DT == "f16":
        return np.float16
    return np.float32


def _build_module(nch, loop_r=None):
    import contextlib
    import concourse.mybir as mybir
    import concourse.tile as tile
    from concourse import bacc

    f32 = mybir.dt.float32
    f16 = mybir.dt.float16
    mmdt = {
        "bf16": mybir.dt.bfloat16,
        "f16": mybir.dt.float16,
        "f32r": mybir.dt.float32r,
    }[DT]
    AF = mybir.ActivationFunctionType

    nc = bacc.Bacc(None, target_bir_lowering=False, enable_asserts=False)
    comb_d = nc.dram_tensor("comb", [nch, 128, COMB_W], mmdt, kind="ExternalInput")
    out_d = nc.dram_tensor("outp", [nch, 128, G, NQ], f16, kind="ExternalOutput")

    with tile.TileContext(nc) as tc:
        with (
            tc.tile_pool(name="big", bufs=8) as big,
            tc.tile_pool(name="work", bufs=5) as work,
            tc.tile_pool(name="ps_s", bufs=2, space="PSUM") as ps_s_pool,
            tc.tile_pool(name="ps_t", bufs=2, space="PSUM") as ps_t_pool,
            tc.tile_pool(name="ps_o", bufs=3, space="PSUM") as ps_o_pool,
            tc.For_i(0, loop_r, 1) if loop_r else contextlib.nullcontext(),
        ):
            for i in range(nch):
                ct = big.tile([128, COMB_W], mmdt, tag="comb")
                # kt/tq/mask half on the SP HWDGE ring, value half on the
                # ACT HWDGE ring: parallel descriptor streams
                nc.sync.dma_start(out=ct[:, :VL_OFF], in_=comb_d[i, :, :VL_OFF])
                nc.scalar.dma_start(out=ct[:, VL_OFF:], in_=comb_d[i, :, VL_OFF:])

                tq_v = ct[:, TQ_OFF : TQ_OFF + G * TQ_W].rearrange(
                    "p (m o c) -> p m o c", m=G, o=HSUB
                )
                mk_v = ct[:, MK_OFF : MK_OFF + MK_W]
                id_v = ct[:CA, ID_OFF : ID_OFF + ID_W]
                kt_v = ct[:, KT_OFF : KT_OFF + G * KT_W].rearrange(
                    "p (m o s) -> p m o s", m=G, o=HSUB
                )
                vl_v = ct[:, VL_OFF : VL_OFF + G * VW].rearrange(
                    "p (m w) -> p m w", m=G
                )

                # scores.T: [CA, G*SUB]; sub m -> columns [m*SUB, (m+1)*SUB)
                ps_s = ps_s_pool.tile([CA, G * SUB], f32)
                for m in range(G):
                    for ho in range(HSUB):
                        nc.tensor.matmul(
                            ps_s[:, m * SUB : (m + 1) * SUB],
                            lhsT=tq_v[:, m, ho, :],
                            rhs=kt_v[:, m, ho, :],
                            start=(ho == 0),
                            stop=(ho == HSUB - 1),
                        )

                sb_e = work.tile([CA, G * SUB], mmdt, tag="exp")
                nc.scalar.activation(out=sb_e, in_=ps_s, func=AF.Exp)

                # transpose exp(scores) to s-on-partitions for the value mm
                ps_t = ps_t_pool.tile([128, G, CA], mmdt)
                for m in range(G):
                    nc.tensor.transpose(
                        ps_t[:, m, :],
                        sb_e[:, m * SUB : (m + 1) * SUB],
                        id_v,
                    )

                al_t = work.tile([128, G, CA], mmdt, tag="alpha")
                nc.vector.tensor_tensor(
                    al_t,
                    ps_t,
                    mk_v[:, :, None].to_broadcast([128, G, CA]),
                    mybir.AluOpType.mult,
                )

                # numerator (+ denominator via ones column at 768) per sub:
                # 4 concurrent PE col-tiles land the [CA, VW] output as
                # [128, NQ] in ONE psum bank, so the PSUM->SBUF copy uses
                # all 128 lanes (4x fewer cycles than a [CA, VW] copy)
                ob = work.tile([128, G, NQ], f16, tag="ob")
                for m in range(G):
                    ps_o = ps_o_pool.tile([128, NQ], f32, tag="ps_o")
                    for j in range(4):
                        nc.tensor.matmul(
                            ps_o[32 * j : 32 * (j + 1), :],
                            lhsT=al_t[:, m, :],
                            rhs=vl_v[:, m, NQ * j : NQ * (j + 1)],
                            start=True,
                            stop=True,
                            tile_position=(0, 32 * j),
                        )
                    if m < G - 1:
                        nc.vector.tensor_copy(out=ob[:, m, :], in_=ps_o)
                    else:
                        nc.scalar.copy(out=ob[:, m, :], in_=ps_o)
                nc.sync.dma_start(out=out_d[i], in_=ob)

    nc.compile()
    return nc


def kernel(key, value, query, seq_len, W, b):
    key = np.ascontiguousarray(np.asarray(key, dtype=np.float32))
    value = np.ascontiguousarray(np.asarray(value, dtype=np.float32))
    query = np.asarray(query, dtype=np.float32)
    W = np.asarray(W, dtype=np.float32)
    bias = np.asarray(b, dtype=np.float32)
    sl = np.asarray(seq_len).astype(np.int64)

    B, S, H_ = key.shape
    assert H_ == H and S % SUB == 0

    # host: tiny projection  tq[b] = tanh(query[b] @ W + bias)  [B, CA, H]
    tq = np.tanh(query.reshape(B * query.shape[1], -1) @ W + bias)
    tq = tq.reshape(B, query.shape[1], H).astype(np.float32)
    npdt = _np_dt()
    tqT_p = {
        bi: np.ascontiguousarray(tq[bi].T.reshape(HSUB, 128, CA)).astype(npdt)
        for bi in range(B)
    }

    # work list: 128-row sub-chunks over valid prefixes
    subs = []  # (batch, s0, nvalid)
    for bi in range(B):
        L = int(sl[bi])
        L = max(1, min(L, S))
        for s0 in range(0, L, SUB):
            subs.append((bi, s0, min(SUB, L - s0)))
    total = len(subs)
    per_core = -(-total // N_CORES)
    nch = -(-per_core // G)

    comb = np.zeros((N_CORES, nch, 128, COMB_W), npdt)
    comb[:, :, :CA, ID_OFF : ID_OFF + ID_W] = np.eye(CA, dtype=np.float32)
    slot_map = [[] for _ in range(N_CORES)]  # per core: list of (slot, m, batch)

    for idx, (bi, s0, nval) in enumerate(subs):
        c = idx % N_CORES
        k = idx // N_CORES
        j, m = k // G, k % G
        row = comb[c, j]
        row[:, TQ_OFF + m * TQ_W : TQ_OFF + (m + 1) * TQ_W] = (
            tqT_p[bi].transpose(1, 0, 2).reshape(128, TQ_W)
        )
        mcol = np.zeros(128, np.float32)
        mcol[:nval] = 1.0
        row[:, MK_OFF + m] = mcol
        kc = key[bi, s0 : s0 + SUB]  # [SUB, H]
        row[:, KT_OFF + m * KT_W : KT_OFF + (m + 1) * KT_W] = (
            kc.T.reshape(HSUB, 128, SUB).transpose(1, 0, 2).reshape(128, KT_W)
        )
        vt = row[:, VL_OFF + m * VW : VL_OFF + (m + 1) * VW]
        vt[:, :H] = value[bi, s0 : s0 + SUB]
        vt[:, H] = 1.0
        slot_map[c].append((j, m, bi))

    if nch not in _module_cache:
        _module_cache[nch] = _build_module(nch)
    nc = _module_cache[nch]

    from concourse.bass_utils import run_bass_kernel_spmd

    in_maps = [{"comb": comb[c]} for c in range(N_CORES)]
    global _last_in_maps
    _last_in_maps = in_maps
    trace = os.environ.get("BASS_KERNEL_TRACE") == "1"
    kwargs = {}
    if trace:
        kwargs = dict(trace=True, trace_cores=list(range(N_CORES)))
    res = run_bass_kernel_spmd(nc, in_maps, core_ids=list(range(N_CORES)), **kwargs)
    if trace and res.exec_time_ns is not None:
        print(f"HW exec time: {res.exec_time_ns} ns")
        print(f"HW exec time mean: {res.mean_exec_time_ns} ns")

    num = np.zeros((B, CA, H), np.float64)
    den = np.zeros((B, CA), np.float64)
    for c in range(N_CORES):
        part = res.results[c]["outp"]  # [nch, 128, G, NQ] col-tiled quarters
        for j, m, bi in slot_map[c]:
            blk = part[j, :, m, :].astype(np.float64).reshape(4, CA, NQ)
            full = np.concatenate(list(blk), axis=1)
            num[bi] += full[:, :H]
            den[bi] += full[:, H]
    out = (num / den[:, :, None]).astype(np.float32)
    return out



# revision 5
# speedup vs baseline: 1.1297x; 1.1297x over previous
"""Trainium2 Bass kernel for ragged-sequence attention (v2: fp8 streams).

Per batch b:
    tq     = tanh(query[b] @ W + bias)                      [CA, H]
    scores = key[b] @ tq.T                                  [S, CA]
    alpha  = exp(scores) ; zeroed value rows mask the tail  [S, CA]
    out[b] = (alpha.T @ value[b]) / alpha.sum(axis=0)[:,None]

Strategy (all-DMA-bytes-bound; the cost model serializes every DMA on one
360 B/ns pipe, so total bytes is the whole game):
  - Raggedness: independent 128-row sub-chunks of each valid prefix; each
    sub yields a partial [CA, 768+1] (col 768 = denominator via a ones
    column in the value tile). Host does the per-batch reduce + divide.
  - Long batches (L >= 600) stream key/value/tq in fp8 e3m4 (key
    pre-scaled x32 to clear the subnormal floor; un-scaled on-device via
    the activation's scale=1/32). Short batches stay fp16 -- quantization
    error scales like sqrt(sum w^2) ~ 1/sqrt(L), so the short batches are
    the accuracy-critical ones and they cost few bytes anyway.
  - Scores come out [s-on-partitions, CA] directly (kt chunk is the
    stationary operand), so there is no transpose, no identity, no mask:
    exp feeds the value matmul as lhsT as-is. Invalid tail rows have
    zeroed value+ones columns, contributing 0 to both numerator and
    denominator regardless of their alpha.
  - fp8 subs are packed two to a "pair" sharing one tq block (pairs hold
    subs of the same batch); pairs/slots are fixed-size so one SPMD
    module serves all 8 cores, light cores padded with zero slots.
  - Input DMAs are dependency-free and issued up front: fp8 pair tiles on
    the SP ring, the fp16 region on the ACT ring; outputs leave on the
    DVE ring (keeping output DMAs off the input queues -- the in-order
    sequencer otherwise stalls input i+1 behind compute i).
"""

import os
import sys

import numpy as np

for _p in ("/opt/trn_rl_repo", "/root/.axon_site/_ro/trn_rl_repo"):
    if os.path.isdir(_p) and _p not in sys.path:
        sys.path.append(_p)

N_CORES = 8
SUB = 128
H = 768
HSUB = H // 128  # 6
CA = 32
VW = 772         # value tile: 768 cols + ones col @768 + pad to 4*193
NQ = VW // 4     # 193
TQW = HSUB * CA  # 192
PAIRW = 2 * H + 2 * VW   # fp8 pair block: kt0 kt1 vt0 vt1
S16W = H + VW            # fp16 slot block: kt vt
KSCALE = 32.0            # fp8 key pre-scale (clears e3m4 subnormal floor)
FP8_MIN_L = 600          # batches at least this long stream in fp8

_module_cache = {}


def _build_module(P8, n16):
    """One SPMD module: P8 fp8 pairs (2 slots each) + n16 fp16 slots."""
    import concourse.mybir as mybir
    import concourse.tile as tile
    from concourse import bacc

    f32 = mybir.dt.float32
    f16 = mybir.dt.float16
    f8 = mybir.dt.float8e3
    AF = mybir.ActivationFunctionType

    nslot = 2 * P8 + n16
    a16w = P8 * TQW + n16 * (TQW + S16W)  # fp8-pair tq blocks, then f16 slots
    TQ16 = P8 * TQW                       # offset of f16-slot region

    nc = bacc.Bacc(None, target_bir_lowering=False, enable_asserts=False)
    a8_d = nc.dram_tensor("a8", [128, P8 * PAIRW], f8, kind="ExternalInput")
    a16_d = nc.dram_tensor("a16", [128, a16w], f16, kind="ExternalInput")
    out_d = nc.dram_tensor("outp", [128, nslot * NQ], f16, kind="ExternalOutput")

    with tile.TileContext(nc) as tc:
        with (
            tc.tile_pool(name="stage", bufs=1) as stage,
            tc.tile_pool(name="ps_s", bufs=3, space="PSUM") as ps_s_pool,
            tc.tile_pool(name="al", bufs=3) as al_pool,
            tc.tile_pool(name="ps_o", bufs=3, space="PSUM") as ps_o_pool,
        ):
            a8_t = [
                stage.tile([128, PAIRW], f8, tag=f"a8_{p}", name=f"a8_{p}")
                for p in range(P8)
            ]
            a16_t = stage.tile([128, a16w], f16, tag="a16", name="a16")
            outsb = stage.tile([128, nslot * NQ], f16, tag="outsb", name="outsb")

            # all input DMAs up front, dependency-free; outputs use a third
            # ring so input issue never queues behind compute
            for p in range(P8):
                nc.sync.dma_start(out=a8_t[p], in_=a8_d[:, p * PAIRW : (p + 1) * PAIRW])
            nc.scalar.dma_start(out=a16_t[:, :TQ16], in_=a16_d[:, :TQ16])
            if n16:
                nc.scalar.dma_start(out=a16_t[:, TQ16:], in_=a16_d[:, TQ16:])

            out_split = (nslot + 1) // 2  # first output DMA once half is done

            def do_slot(slot, kt_v, vt_v, tq_v, scale):
                ps_s = ps_s_pool.tile([128, CA], f32, tag="ps_s")
                for ho in range(HSUB):
                    nc.tensor.matmul(
                        ps_s,
                        lhsT=kt_v[:, ho * 128 : (ho + 1) * 128],
                        rhs=tq_v[:, ho * CA : (ho + 1) * CA],
                        start=(ho == 0),
                        stop=(ho == HSUB - 1),
                    )
                al = al_pool.tile([128, CA], f16, tag="al")
                nc.scalar.activation(out=al, in_=ps_s, func=AF.Exp, scale=scale)
                ps_o = ps_o_pool.tile([128, NQ], f32, tag="ps_o")
                for j in range(4):
                    nc.tensor.matmul(
                        ps_o[32 * j : 32 * (j + 1), :],
                        lhsT=al,
                        rhs=vt_v[:, j * NQ : (j + 1) * NQ],
                        start=True,
                        stop=True,
                        tile_position=(0, 32 * j),
                    )
                if slot % 2 == 0:
                    nc.vector.tensor_copy(out=outsb[:, slot * NQ : (slot + 1) * NQ], in_=ps_o)
                else:
                    nc.scalar.copy(out=outsb[:, slot * NQ : (slot + 1) * NQ], in_=ps_o)
                if slot + 1 == out_split:
                    nc.gpsimd.dma_start(
                        out=out_d[:, : out_split * NQ],
                        in_=outsb[:, : out_split * NQ],
                    )

            slot = 0
            for p in range(P8):
                for m in range(2):
                    do_slot(
                        slot,
                        a8_t[p][:, m * H : (m + 1) * H],
                        a8_t[p][:, 2 * H + m * VW : 2 * H + (m + 1) * VW],
                        a16_t[:, p * TQW : (p + 1) * TQW],
                        1.0 / KSCALE,
                    )
                    slot += 1
            for k in range(n16):
                base = TQ16 + k * (TQW + S16W)
                do_slot(
                    slot,
                    a16_t[:, base + TQW : base + TQW + H],
                    a16_t[:, base + TQW + H : base + TQW + H + VW],
                    a16_t[:, base : base + TQW],
                    1.0,
                )
                slot += 1

            nc.gpsimd.dma_start(
                out=out_d[:, out_split * NQ :], in_=outsb[:, out_split * NQ :]
            )

    nc.compile()
    return nc


def kernel(key, value, query, seq_len, W, b):
    import ml_dtypes

    e3 = ml_dtypes.float8_e3m4
    key = np.ascontiguousarray(np.asarray(key, dtype=np.float32))
    value = np.ascontiguousarray(np.asarray(value, dtype=np.float32))
    query = np.asarray(query, dtype=np.float32)
    W = np.asarray(W, dtype=np.float32)
    bias = np.asarray(b, dtype=np.float32)
    sl = np.asarray(seq_len).astype(np.int64)

    B, S, H_ = key.shape
    assert H_ == H and S % SUB == 0

    # host: tiny projection  tq[b] = tanh(query[b] @ W + bias)  [B, CA, H]
    tq = np.tanh(query.reshape(B * query.shape[1], -1) @ W + bias)
    tq = tq.reshape(B, query.shape[1], H).astype(np.float32)
    # [128, 192] chunk-major transposed layout per batch
    tqT = {
        bi: np.ascontiguousarray(
            tq[bi].T.reshape(HSUB, 128, CA).transpose(1, 0, 2).reshape(128, TQW)
        )
        for bi in range(B)
    }

    # work lists: 128-row sub-chunks of each valid prefix
    subs8, subs16 = [], []  # (batch, s0, nvalid)
    for bi in range(B):
        L = max(1, min(int(sl[bi]), S))
        lst = subs8 if L >= FP8_MIN_L else subs16
        for s0 in range(0, L, SUB):
            lst.append((bi, s0, min(SUB, L - s0)))

    # fp8 subs -> same-batch pairs (a pair shares one tq block)
    pairs = []
    i = 0
    while i < len(subs8):
        if i + 1 < len(subs8) and subs8[i][0] == subs8[i + 1][0]:
            pairs.append([subs8[i], subs8[i + 1]])
            i += 2
        else:
            pairs.append([subs8[i]])
            i += 1

    P8 = -(-len(pairs) // N_CORES) if pairs else 0
    n16 = -(-len(subs16) // N_CORES) if subs16 else 0
    nslot = 2 * P8 + n16
    a16w = P8 * TQW + n16 * (TQW + S16W)
    TQ16 = P8 * TQW

    a8 = np.zeros((N_CORES, 128, max(P8 * PAIRW, 1)), e3)
    a16 = np.zeros((N_CORES, 128, max(a16w, 1)), np.float16)
    slot_map = [[] for _ in range(N_CORES)]  # per core: (slot, batch)

    def pack_kt(bi, s0, nval, scale):
        kc = key[bi, s0 : s0 + SUB].copy()
        kc[nval:] = 0.0
        return (kc.T * scale).reshape(HSUB, 128, SUB).transpose(1, 0, 2).reshape(128, H)

    def pack_vt(bi, s0, nval):
        vt = np.zeros((128, VW), np.float32)
        vt[:nval, :H] = value[bi, s0 : s0 + nval]
        vt[:nval, H] = 1.0
        return vt

    for pi, pair in enumerate(pairs):
        c, p = pi % N_CORES, pi // N_CORES
        a16[c, :, p * TQW : (p + 1) * TQW] = tqT[pair[0][0]].astype(np.float16)
        for m, (bi, s0, nval) in enumerate(pair):
            a8[c, :, p * PAIRW + m * H : p * PAIRW + (m + 1) * H] = pack_kt(
                bi, s0, nval, KSCALE
            ).astype(e3)
            a8[
                c, :, p * PAIRW + 2 * H + m * VW : p * PAIRW + 2 * H + (m + 1) * VW
            ] = pack_vt(bi, s0, nval).astype(e3)
            slot_map[c].append((2 * p + m, bi))

    for si, (bi, s0, nval) in enumerate(subs16):
        c, k = si % N_CORES, si // N_CORES
        base = TQ16 + k * (TQW + S16W)
        a16[c, :, base : base + TQW] = tqT[bi].astype(np.float16)
        a16[c, :, base + TQW : base + TQW + H] = pack_kt(bi, s0, nval, 1.0)
        a16[c, :, base + TQW + H : base + TQW + H + VW] = pack_vt(bi, s0, nval)
        slot_map[c].append((2 * P8 + k, bi))

    cfg = (P8, n16)
    if cfg not in _module_cache:
        _module_cache[cfg] = _build_module(P8, n16)
    nc = _module_cache[cfg]

    from concourse.bass_utils import run_bass_kernel_spmd

    in_maps = [{"a8": a8[c], "a16": a16[c]} for c in range(N_CORES)]
    trace = os.environ.get("BASS_KERNEL_TRACE") == "1"
    kwargs = {}
    if trace:
        kwargs = dict(trace=True, trace_cores=list(range(N_CORES)))
    res = run_bass_kernel_spmd(nc, in_maps, core_ids=list(range(N_CORES)), **kwargs)
    if trace and res.exec_time_ns is not None:
        print(f"HW exec time: {res.exec_time_ns} ns")
        print(f"HW exec time mean: {res.mean_exec_time_ns} ns")

    num = np.zeros((B, CA, H), np.float64)
    den = np.zeros((B, CA), np.float64)
    for c in range(N_CORES):
        part = res.results[c]["outp"]  # [128, nslot*NQ]; 4 col-tiled quarters
        for slot, bi in slot_map[c]:
            blk = part[:, slot * NQ : (slot + 1) * NQ].astype(np.float64)
            full = np.concatenate(list(blk.reshape(4, CA, NQ)), axis=1)  # [CA, VW]
            num[bi] += full[:, :H]
            den[bi] += full[:, H]
    out = (num / den[:, :, None]).astype(np.float32)
    return out


# revision 7
# speedup vs baseline: 1.2373x; 1.0953x over previous
"""Trainium2 Bass kernel for ragged-sequence attention (v3: fp8 + pipelined PE).

Per batch b:
    tq     = tanh(query[b] @ W + bias)                      [CA, H]
    scores = key[b] @ tq.T                                  [S, CA]
    alpha  = exp(scores) ; zeroed value rows mask the tail  [S, CA]
    out[b] = (alpha.T @ value[b]) / alpha.sum(axis=0)[:,None]

Strategy (all-DMA-bytes-bound; the cost model serializes every DMA on one
360 B/ns pipe, so total bytes is the whole game):
  - Raggedness: independent 128-row sub-chunks of each valid prefix; each
    sub yields a partial [CA, 768+1] (col 768 = denominator via a ones
    column in the value tile). Host does the per-batch reduce + divide.
  - Long batches (L >= 600) stream key/value/tq in fp8 e3m4 (key
    pre-scaled x32 to clear the subnormal floor; un-scaled on-device via
    the activation's scale=1/32). Short batches stay fp16 -- quantization
    error scales like sqrt(sum w^2) ~ 1/sqrt(L), so the short batches are
    the accuracy-critical ones and they cost few bytes anyway.
  - Scores come out [s-on-partitions, CA] directly (kt chunk is the
    stationary operand), so there is no transpose, no identity, no mask:
    exp feeds the value matmul as lhsT as-is. Invalid tail rows have
    zeroed value+ones columns, contributing 0 to both numerator and
    denominator regardless of their alpha.
  - fp8 subs are packed two to a "pair" (same batch) sharing one tq block
    and one PSUM output accumulator; pairs/slots are fixed-size so one
    SPMD module serves all 8 cores, light cores padded with zero slots.
  - The PE stream is software-pipelined two slots deep (scores of slot
    i+2 are emitted before value-matmuls of slot i) so the in-order PE
    sequencer never parks on the exp latency between the two matmuls.
  - Input DMAs are dependency-free and issued up front: fp8 pair tiles on
    the SP ring, the fp16 region on the ACT ring; outputs leave on the SP
    ring after the inputs (keeping output DMAs off the compute-facing
    queues -- an in-order sequencer otherwise stalls input i+1 behind
    compute i).
"""

import os
import sys

import numpy as np

for _p in ("/opt/trn_rl_repo", "/root/.axon_site/_ro/trn_rl_repo"):
    if os.path.isdir(_p) and _p not in sys.path:
        sys.path.append(_p)

N_CORES = 8
SUB = 128
H = 768
HSUB = H // 128  # 6
CA = 32
VW = 772         # value tile: 768 cols + ones col @768 + pad to 4*193
NQ = VW // 4     # 193
TQW = HSUB * CA  # 192
PAIRW = 2 * H + TQW + 2 * VW  # fp8 pair block: kt0 kt1 tq vt0 vt1 (e3m4)
S16W = TQW + H + VW           # fp16 slot block: tq kt vt
KSCALE = 32.0            # fp8 key pre-scale (clears e3m4 subnormal floor)
FP8_MIN_L = 600          # batches at least this long stream in fp8

_module_cache = {}


def _build_module(P8, n16):
    """One SPMD module: P8 fp8 pairs (2 slots each) + n16 fp16 slots."""
    import concourse.mybir as mybir
    import concourse.tile as tile
    from concourse import bacc

    f32 = mybir.dt.float32
    f16 = mybir.dt.float16
    f8 = mybir.dt.float8e3
    AF = mybir.ActivationFunctionType

    nout = P8 + n16  # one output block per pair + per f16 slot

    nc = bacc.Bacc(None, target_bir_lowering=False, enable_asserts=False)
    a8_d = nc.dram_tensor("a8", [128, P8 * PAIRW], f8, kind="ExternalInput")
    a16_d = nc.dram_tensor("a16", [128, max(n16 * S16W, 1)], f16, kind="ExternalInput")
    out_d = nc.dram_tensor("outp", [128, nout * NQ], f16, kind="ExternalOutput")

    with tile.TileContext(nc) as tc:
        with (
            tc.tile_pool(name="stage", bufs=1) as stage,
            tc.tile_pool(name="ps_s", bufs=3, space="PSUM") as ps_s_pool,
            tc.tile_pool(name="al", bufs=4) as al_pool,
            tc.tile_pool(name="ps_o", bufs=3, space="PSUM") as ps_o_pool,
        ):
            a8_t = [
                stage.tile([128, PAIRW], f8, tag=f"a8_{p}", name=f"a8_{p}")
                for p in range(P8)
            ]
            a16_t = (
                stage.tile([128, n16 * S16W], f16, tag="a16", name="a16")
                if n16
                else None
            )
            outsb = stage.tile([128, nout * NQ], f16, tag="outsb", name="outsb")

            # all input DMAs up front, dependency-free
            for p in range(P8):
                nc.sync.dma_start(out=a8_t[p], in_=a8_d[:, p * PAIRW : (p + 1) * PAIRW])
            if n16:
                half = (n16 // 2) * S16W
                if half:
                    nc.scalar.dma_start(out=a16_t[:, :half], in_=a16_d[:, :half])
                nc.scalar.dma_start(out=a16_t[:, half:], in_=a16_d[:, half:])

            # slot descriptors: (kt view, vt view, tq view, exp scale,
            #                    out block index, psum start, psum stop)
            slots = []
            for p in range(P8):
                t = a8_t[p]
                for m in range(2):
                    slots.append(
                        (
                            t[:, m * H : (m + 1) * H],
                            t[:, 2 * H + TQW + m * VW : 2 * H + TQW + (m + 1) * VW],
                            t[:, 2 * H : 2 * H + TQW],
                            1.0 / KSCALE,
                            p,
                            m == 0,
                            m == 1,
                        )
                    )
            for k in range(n16):
                base = k * S16W
                slots.append(
                    (
                        a16_t[:, base + TQW : base + TQW + H],
                        a16_t[:, base + TQW + H : base + TQW + H + VW],
                        a16_t[:, base : base + TQW],
                        1.0,
                        P8 + k,
                        True,
                        True,
                    )
                )
            n = len(slots)

            ps_s_t = [None] * n
            al_t = [None] * n
            ps_o_t = {}

            def emit_scores(i):
                kt_v, _, tq_v, _, _, _, _ = slots[i]
                ps_s = ps_s_pool.tile([128, CA], f32, tag="ps_s")
                for ho in range(HSUB):
                    nc.tensor.matmul(
                        ps_s,
                        lhsT=kt_v[:, ho * 128 : (ho + 1) * 128],
                        rhs=tq_v[:, ho * CA : (ho + 1) * CA],
                        start=(ho == 0),
                        stop=(ho == HSUB - 1),
                    )
                al = al_pool.tile([128, CA], f16, tag="al")
                nc.scalar.activation(out=al, in_=ps_s, func=AF.Exp, scale=slots[i][3])
                ps_s_t[i], al_t[i] = ps_s, al

            out_written = 0

            def emit_value(i):
                nonlocal out_written
                _, vt_v, _, _, ob, ps_start, ps_stop = slots[i]
                if ps_start:
                    ps_o_t[ob] = ps_o_pool.tile(
                        [128, NQ], f32, tag="ps_o", name=f"ps_o_{ob}"
                    )
                ps_o = ps_o_t[ob]
                for j in range(4):
                    nc.tensor.matmul(
                        ps_o[32 * j : 32 * (j + 1), :],
                        lhsT=al_t[i],
                        rhs=vt_v[:, j * NQ : (j + 1) * NQ],
                        start=ps_start,
                        stop=ps_stop,
                        tile_position=(0, 32 * j),
                    )
                if ps_stop:
                    nc.vector.tensor_copy(
                        out=outsb[:, ob * NQ : (ob + 1) * NQ], in_=ps_o
                    )
                    # stream finished output blocks out in ~quarter pieces
                    done = ob + 1
                    if done - out_written >= max(2, nout // 4) and done < nout:
                        nc.sync.dma_start(
                            out=out_d[:, out_written * NQ : done * NQ],
                            in_=outsb[:, out_written * NQ : done * NQ],
                        )
                        out_written = done

            # two-slot-deep software pipeline on the PE stream
            emit_scores(0)
            if n > 1:
                emit_scores(1)
            for i in range(n):
                emit_value(i)
                if i + 2 < n:
                    emit_scores(i + 2)

            nc.sync.dma_start(
                out=out_d[:, out_written * NQ :], in_=outsb[:, out_written * NQ :]
            )

    nc.compile()
    return nc


def kernel(key, value, query, seq_len, W, b):
    import ml_dtypes

    e3 = ml_dtypes.float8_e3m4
    key = np.ascontiguousarray(np.asarray(key, dtype=np.float32))
    value = np.ascontiguousarray(np.asarray(value, dtype=np.float32))
    query = np.asarray(query, dtype=np.float32)
    W = np.asarray(W, dtype=np.float32)
    bias = np.asarray(b, dtype=np.float32)
    sl = np.asarray(seq_len).astype(np.int64)

    B, S, H_ = key.shape
    assert H_ == H and S % SUB == 0

    # host: tiny projection  tq[b] = tanh(query[b] @ W + bias)  [B, CA, H]
    tq = np.tanh(query.reshape(B * query.shape[1], -1) @ W + bias)
    tq = tq.reshape(B, query.shape[1], H).astype(np.float32)
    # [128, 192] chunk-major transposed layout per batch
    tqT = {
        bi: np.ascontiguousarray(
            tq[bi].T.reshape(HSUB, 128, CA).transpose(1, 0, 2).reshape(128, TQW)
        )
        for bi in range(B)
    }

    # work lists: 128-row sub-chunks of each valid prefix
    subs8, subs16 = [], []  # (batch, s0, nvalid)
    for bi in range(B):
        L = max(1, min(int(sl[bi]), S))
        lst = subs8 if L >= FP8_MIN_L else subs16
        for s0 in range(0, L, SUB):
            lst.append((bi, s0, min(SUB, L - s0)))

    # fp8 subs -> same-batch pairs (a pair shares tq + output accumulator)
    pairs = []
    i = 0
    while i < len(subs8):
        if i + 1 < len(subs8) and subs8[i][0] == subs8[i + 1][0]:
            pairs.append([subs8[i], subs8[i + 1]])
            i += 2
        else:
            pairs.append([subs8[i]])
            i += 1

    P8 = -(-len(pairs) // N_CORES) if pairs else 0
    n16 = -(-len(subs16) // N_CORES) if subs16 else 0
    nout = P8 + n16

    a8 = np.zeros((N_CORES, 128, max(P8 * PAIRW, 1)), e3)
    a16 = np.zeros((N_CORES, 128, max(n16 * S16W, 1)), np.float16)
    out_map = [[] for _ in range(N_CORES)]  # per core: (out block, batch)

    def pack_kt(bi, s0, nval, scale):
        kc = key[bi, s0 : s0 + SUB].copy()
        kc[nval:] = 0.0
        return (kc.T * scale).reshape(HSUB, 128, SUB).transpose(1, 0, 2).reshape(128, H)

    def pack_vt(bi, s0, nval):
        vt = np.zeros((128, VW), np.float32)
        vt[:nval, :H] = value[bi, s0 : s0 + nval]
        vt[:nval, H] = 1.0
        return vt

    for pi, pair in enumerate(pairs):
        c, p = pi % N_CORES, pi // N_CORES
        blk = a8[c, :, p * PAIRW : (p + 1) * PAIRW]
        blk[:, 2 * H : 2 * H + TQW] = tqT[pair[0][0]].astype(e3)
        for m, (bi, s0, nval) in enumerate(pair):
            blk[:, m * H : (m + 1) * H] = pack_kt(bi, s0, nval, KSCALE).astype(e3)
            blk[:, 2 * H + TQW + m * VW : 2 * H + TQW + (m + 1) * VW] = pack_vt(
                bi, s0, nval
            ).astype(e3)
        out_map[c].append((p, pair[0][0]))

    for si, (bi, s0, nval) in enumerate(subs16):
        c, k = si % N_CORES, si // N_CORES
        base = k * S16W
        a16[c, :, base : base + TQW] = tqT[bi].astype(np.float16)
        a16[c, :, base + TQW : base + TQW + H] = pack_kt(bi, s0, nval, 1.0)
        a16[c, :, base + TQW + H : base + TQW + H + VW] = pack_vt(bi, s0, nval)
        out_map[c].append((P8 + k, bi))

    cfg = (P8, n16)
    if cfg not in _module_cache:
        _module_cache[cfg] = _build_module(P8, n16)
    nc = _module_cache[cfg]

    from concourse.bass_utils import run_bass_kernel_spmd

    in_maps = [{"a8": a8[c], "a16": a16[c]} for c in range(N_CORES)]
    trace = os.environ.get("BASS_KERNEL_TRACE") == "1"
    kwargs = {}
    if trace:
        kwargs = dict(trace=True, trace_cores=list(range(N_CORES)))
    res = run_bass_kernel_spmd(nc, in_maps, core_ids=list(range(N_CORES)), **kwargs)
    if trace and res.exec_time_ns is not None:
        print(f"HW exec time: {res.exec_time_ns} ns")
        print(f"HW exec time mean: {res.mean_exec_time_ns} ns")

    num = np.zeros((B, CA, H), np.float64)
    den = np.zeros((B, CA), np.float64)
    for c in range(N_CORES):
        part = res.results[c]["outp"]  # [128, nout*NQ]; 4 col-tiled quarters
        for ob, bi in out_map[c]:
            blk = part[:, ob * NQ : (ob + 1) * NQ].astype(np.float64)
            full = np.concatenate(list(blk.reshape(4, CA, NQ)), axis=1)  # [CA, VW]
            num[bi] += full[:, :H]
            den[bi] += full[:, H]
    out = (num / den[:, :, None]).astype(np.float32)
    return out


# revision 9
# speedup vs baseline: 1.3176x; 1.0648x over previous
"""Trainium2 Bass kernel for ragged-sequence attention (v3: fp8 + pipelined PE).

Per batch b:
    tq     = tanh(query[b] @ W + bias)                      [CA, H]
    scores = key[b] @ tq.T                                  [S, CA]
    alpha  = exp(scores) ; zeroed value rows mask the tail  [S, CA]
    out[b] = (alpha.T @ value[b]) / alpha.sum(axis=0)[:,None]

Strategy (all-DMA-bytes-bound; the cost model serializes every DMA on one
360 B/ns pipe, so total bytes is the whole game):
  - Raggedness: independent 128-row sub-chunks of each valid prefix; each
    sub yields a partial [CA, 768+1] (col 768 = denominator via a ones
    column in the value tile). Host does the per-batch reduce + divide.
  - Long batches (L >= 600) stream key/value/tq in fp8 e3m4 (key
    pre-scaled x32 to clear the subnormal floor; un-scaled on-device via
    the activation's scale=1/32). Short batches stay fp16 -- quantization
    error scales like sqrt(sum w^2) ~ 1/sqrt(L), so the short batches are
    the accuracy-critical ones and they cost few bytes anyway.
  - Scores come out [s-on-partitions, CA] directly (kt chunk is the
    stationary operand), so there is no transpose, no identity, no mask:
    exp feeds the value matmul as lhsT as-is. Invalid tail rows have
    zeroed value+ones columns, contributing 0 to both numerator and
    denominator regardless of their alpha.
  - fp8 subs are packed two to a "pair" (same batch) sharing one tq block
    and one PSUM output accumulator; pairs/slots are fixed-size so one
    SPMD module serves all 8 cores, light cores padded with zero slots.
  - The PE stream is software-pipelined two slots deep (scores of slot
    i+2 are emitted before value-matmuls of slot i) so the in-order PE
    sequencer never parks on the exp latency between the two matmuls.
  - Input DMAs are dependency-free and issued up front: fp8 pair tiles on
    the SP ring, the fp16 region on the ACT ring; outputs leave on the SP
    ring after the inputs (keeping output DMAs off the compute-facing
    queues -- an in-order sequencer otherwise stalls input i+1 behind
    compute i).
"""

import os
import sys

import numpy as np

for _p in ("/opt/trn_rl_repo", "/root/.axon_site/_ro/trn_rl_repo"):
    if os.path.isdir(_p) and _p not in sys.path:
        sys.path.append(_p)

N_CORES = 8
SUB = 128
H = 768
HSUB = H // 128  # 6
CA = 32
VW = 772         # value tile: 768 cols + ones col @768 + pad to 4*193
NQ = VW // 4     # 193
TQW = HSUB * CA  # 192
PAIRW = 2 * H + TQW + 2 * VW  # fp8 pair block: kt0 kt1 tq vt0 vt1 (e3m4)
S16W = TQW + H + VW           # fp16 slot block: tq kt vt
KSCALE = 32.0            # fp8 key pre-scale (clears e3m4 subnormal floor)
FP8_MIN_L = 600          # batches at least this long stream in fp8

_module_cache = {}


def _build_module(P8, n16):
    """One SPMD module: P8 fp8 pairs (2 slots each) + n16 fp16 slots."""
    import concourse.mybir as mybir
    import concourse.tile as tile
    from concourse import bacc

    f32 = mybir.dt.float32
    f16 = mybir.dt.float16
    f8 = mybir.dt.float8e3
    AF = mybir.ActivationFunctionType

    nout = P8 + n16  # one output block per pair + per f16 slot

    nc = bacc.Bacc(None, target_bir_lowering=False, enable_asserts=False)
    a8_d = nc.dram_tensor("a8", [128, P8 * PAIRW], f8, kind="ExternalInput")
    a16_d = nc.dram_tensor("a16", [128, max(n16 * S16W, 1)], f16, kind="ExternalInput")
    out_d = nc.dram_tensor("outp", [128, nout * NQ], f16, kind="ExternalOutput")

    with tile.TileContext(nc) as tc:
        with (
            tc.tile_pool(name="stage", bufs=1) as stage,
            tc.tile_pool(name="ps_s", bufs=4, space="PSUM") as ps_s_pool,
            tc.tile_pool(name="al", bufs=6) as al_pool,
            tc.tile_pool(name="ps_o", bufs=3, space="PSUM") as ps_o_pool,
        ):
            a8_t = [
                stage.tile([128, PAIRW], f8, tag=f"a8_{p}", name=f"a8_{p}")
                for p in range(P8)
            ]
            a16_t = (
                stage.tile([128, n16 * S16W], f16, tag="a16", name="a16")
                if n16
                else None
            )
            outsb = stage.tile([128, nout * NQ], f16, tag="outsb", name="outsb")

            # all input DMAs up front on one ring, in compute order; the
            # first pair is split kt+tq / vt so scores can start early; the
            # ACT ring carries no DMAs so exp latency stays clean
            KTQ = 2 * H + TQW
            for p in range(P8):
                if p == 0:
                    nc.sync.dma_start(out=a8_t[0][:, :KTQ], in_=a8_d[:, :KTQ])
                    nc.sync.dma_start(out=a8_t[0][:, KTQ:], in_=a8_d[:, KTQ:PAIRW])
                else:
                    nc.sync.dma_start(
                        out=a8_t[p], in_=a8_d[:, p * PAIRW : (p + 1) * PAIRW]
                    )
            if n16:
                half = max(n16 - 1, 1) * S16W
                nc.sync.dma_start(out=a16_t[:, :half], in_=a16_d[:, :half])
                if half < n16 * S16W:
                    nc.sync.dma_start(out=a16_t[:, half:], in_=a16_d[:, half:])

            # slot descriptors: (kt view, vt view, tq view, exp scale)
            slots = []
            groups = []  # lists of slot indices sharing one psum/out block
            for p in range(P8):
                t = a8_t[p]
                for m in range(2):
                    slots.append(
                        (
                            t[:, m * H : (m + 1) * H],
                            t[:, 2 * H + TQW + m * VW : 2 * H + TQW + (m + 1) * VW],
                            t[:, 2 * H : 2 * H + TQW],
                            1.0 / KSCALE,
                        )
                    )
                groups.append([2 * p, 2 * p + 1])
            for k in range(n16):
                base = k * S16W
                slots.append(
                    (
                        a16_t[:, base + TQW : base + TQW + H],
                        a16_t[:, base + TQW + H : base + TQW + H + VW],
                        a16_t[:, base : base + TQW],
                        1.0,
                    )
                )
                groups.append([2 * P8 + k])
            ng = len(groups)

            al_t = [None] * len(slots)

            def emit_scores(g):
                for i in groups[g]:
                    kt_v, _, tq_v, scale = slots[i]
                    ps_s = ps_s_pool.tile([128, CA], f32, tag="ps_s")
                    for ho in range(HSUB):
                        nc.tensor.matmul(
                            ps_s,
                            lhsT=kt_v[:, ho * 128 : (ho + 1) * 128],
                            rhs=tq_v[:, ho * CA : (ho + 1) * CA],
                            start=(ho == 0),
                            stop=(ho == HSUB - 1),
                        )
                    al = al_pool.tile([128, CA], f16, tag="al")
                    nc.scalar.activation(out=al, in_=ps_s, func=AF.Exp, scale=scale)
                    al_t[i] = al

            out_written = 0

            def emit_value(g):
                nonlocal out_written
                ps_o = ps_o_pool.tile([128, NQ], f32, tag="ps_o", name=f"ps_o_{g}")
                for gi, i in enumerate(groups[g]):
                    vt_v = slots[i][1]
                    for j in range(4):
                        nc.tensor.matmul(
                            ps_o[32 * j : 32 * (j + 1), :],
                            lhsT=al_t[i],
                            rhs=vt_v[:, j * NQ : (j + 1) * NQ],
                            start=(gi == 0),
                            stop=(gi == len(groups[g]) - 1),
                            tile_position=(0, 32 * j),
                        )
                nc.vector.tensor_copy(out=outsb[:, g * NQ : (g + 1) * NQ], in_=ps_o)
                done = g + 1
                if done - out_written >= 2 and done < ng:
                    nc.sync.dma_start(
                        out=out_d[:, out_written * NQ : done * NQ],
                        in_=outsb[:, out_written * NQ : done * NQ],
                    )
                    out_written = done

            # one-group-deep software pipeline on the PE stream; value
            # matmuls of a group stay contiguous (the ldweights pass keeps
            # psum accumulation groups together, so emit them that way)
            emit_scores(0)
            if ng > 1:
                emit_scores(1)
            for g in range(ng):
                emit_value(g)
                if g + 2 < ng:
                    emit_scores(g + 2)

            nc.sync.dma_start(
                out=out_d[:, out_written * NQ :], in_=outsb[:, out_written * NQ :]
            )

    nc.compile()
    return nc


def kernel(key, value, query, seq_len, W, b):
    import ml_dtypes

    e3 = ml_dtypes.float8_e3m4
    key = np.ascontiguousarray(np.asarray(key, dtype=np.float32))
    value = np.ascontiguousarray(np.asarray(value, dtype=np.float32))
    query = np.asarray(query, dtype=np.float32)
    W = np.asarray(W, dtype=np.float32)
    bias = np.asarray(b, dtype=np.float32)
    sl = np.asarray(seq_len).astype(np.int64)

    B, S, H_ = key.shape
    assert H_ == H and S % SUB == 0

    # host: tiny projection  tq[b] = tanh(query[b] @ W + bias)  [B, CA, H]
    tq = np.tanh(query.reshape(B * query.shape[1], -1) @ W + bias)
    tq = tq.reshape(B, query.shape[1], H).astype(np.float32)
    # [128, 192] chunk-major transposed layout per batch
    tqT = {
        bi: np.ascontiguousarray(
            tq[bi].T.reshape(HSUB, 128, CA).transpose(1, 0, 2).reshape(128, TQW)
        )
        for bi in range(B)
    }

    # work lists: 128-row sub-chunks of each valid prefix
    subs8, subs16 = [], []  # (batch, s0, nvalid)
    for bi in range(B):
        L = max(1, min(int(sl[bi]), S))
        lst = subs8 if L >= FP8_MIN_L else subs16
        for s0 in range(0, L, SUB):
            lst.append((bi, s0, min(SUB, L - s0)))

    # fp8 subs -> same-batch pairs (a pair shares tq + output accumulator)
    pairs = []
    i = 0
    while i < len(subs8):
        if i + 1 < len(subs8) and subs8[i][0] == subs8[i + 1][0]:
            pairs.append([subs8[i], subs8[i + 1]])
            i += 2
        else:
            pairs.append([subs8[i]])
            i += 1

    P8 = -(-len(pairs) // N_CORES) if pairs else 0
    n16 = -(-len(subs16) // N_CORES) if subs16 else 0
    nout = P8 + n16

    a8 = np.zeros((N_CORES, 128, max(P8 * PAIRW, 1)), e3)
    a16 = np.zeros((N_CORES, 128, max(n16 * S16W, 1)), np.float16)
    out_map = [[] for _ in range(N_CORES)]  # per core: (out block, batch)

    def pack_kt(bi, s0, nval, scale):
        kc = key[bi, s0 : s0 + SUB].copy()
        kc[nval:] = 0.0
        return (kc.T * scale).reshape(HSUB, 128, SUB).transpose(1, 0, 2).reshape(128, H)

    def pack_vt(bi, s0, nval):
        vt = np.zeros((128, VW), np.float32)
        vt[:nval, :H] = value[bi, s0 : s0 + nval]
        vt[:nval, H] = 1.0
        return vt

    for pi, pair in enumerate(pairs):
        c, p = pi % N_CORES, pi // N_CORES
        blk = a8[c, :, p * PAIRW : (p + 1) * PAIRW]
        blk[:, 2 * H : 2 * H + TQW] = tqT[pair[0][0]].astype(e3)
        for m, (bi, s0, nval) in enumerate(pair):
            blk[:, m * H : (m + 1) * H] = pack_kt(bi, s0, nval, KSCALE).astype(e3)
            blk[:, 2 * H + TQW + m * VW : 2 * H + TQW + (m + 1) * VW] = pack_vt(
                bi, s0, nval
            ).astype(e3)
        out_map[c].append((p, pair[0][0]))

    for si, (bi, s0, nval) in enumerate(subs16):
        c, k = si % N_CORES, si // N_CORES
        base = k * S16W
        a16[c, :, base : base + TQW] = tqT[bi].astype(np.float16)
        a16[c, :, base + TQW : base + TQW + H] = pack_kt(bi, s0, nval, 1.0)
        a16[c, :, base + TQW + H : base + TQW + H + VW] = pack_vt(bi, s0, nval)
        out_map[c].append((P8 + k, bi))

    cfg = (P8, n16)
    if cfg not in _module_cache:
        _module_cache[cfg] = _build_module(P8, n16)
    nc = _module_cache[cfg]

    from concourse.bass_utils import run_bass_kernel_spmd

    in_maps = [{"a8": a8[c], "a16": a16[c]} for c in range(N_CORES)]
    trace = os.environ.get("BASS_KERNEL_TRACE") == "1"
    kwargs = {}
    if trace:
        kwargs = dict(trace=True, trace_cores=list(range(N_CORES)))
    res = run_bass_kernel_spmd(nc, in_maps, core_ids=list(range(N_CORES)), **kwargs)
    if trace and res.exec_time_ns is not None:
        print(f"HW exec time: {res.exec_time_ns} ns")
        print(f"HW exec time mean: {res.mean_exec_time_ns} ns")

    num = np.zeros((B, CA, H), np.float64)
    den = np.zeros((B, CA), np.float64)
    for c in range(N_CORES):
        part = res.results[c]["outp"]  # [128, nout*NQ]; 4 col-tiled quarters
        for ob, bi in out_map[c]:
            blk = part[:, ob * NQ : (ob + 1) * NQ].astype(np.float64)
            full = np.concatenate(list(blk.reshape(4, CA, NQ)), axis=1)  # [CA, VW]
            num[bi] += full[:, :H]
            den[bi] += full[:, H]
    out = (num / den[:, :, None]).astype(np.float32)
    return out


# revision 10
# speedup vs baseline: 1.4427x; 1.0950x over previous
"""Trainium2 Bass kernel for ragged-sequence attention (v5).

Per batch b:
    tq     = tanh(query[b] @ W + bias)                      [CA, H]
    scores = key[b] @ tq.T                                  [S, CA]
    alpha  = exp(scores) ; zeroed value rows mask the tail  [S, CA]
    out[b] = (alpha.T @ value[b]) / alpha.sum(axis=0)[:,None]

Strategy (all-DMA-bytes-bound; the cost model serializes every DMA on one
360 B/ns pipe, so total bytes is the whole game):
  - Raggedness: independent 128-row sub-chunks of each valid prefix; each
    sub yields a partial [CA, 768+1] (col 768 = denominator via a ones
    column in the value tile). Host does the per-batch reduce + divide.
  - Batches with L >= 300 stream key/value/tq in fp8 e3m4 (key pre-scaled
    x32 to clear the subnormal floor; un-scaled on-device via the exp's
    scale=1/32). Short batches stay fp16 -- quantization error scales
    like sqrt(sum w^2) ~ 1/sqrt(L), so the shortest batches are the
    accuracy-critical ones and they cost few bytes anyway.
  - Scores come out [s-on-partitions, CA] directly (kt chunk is the
    stationary operand), so there is no transpose, no identity, no mask:
    exp feeds the value matmul as lhsT as-is. Invalid tail rows have
    zeroed value+ones columns, contributing 0 to both numerator and
    denominator regardless of their alpha.
  - fp8 subs are packed two to a "pair" (same batch) sharing one tq block
    and one PSUM output accumulator; pairs/slots are fixed-size so one
    SPMD module serves all 8 cores, light cores padded with zero slots.
  - The PE stream is software-pipelined two groups deep (scores of group
    g+3 are emitted after value-matmuls of group g) so the in-order PE
    sequencer hides the exp latency; value matmuls of a group stay
    contiguous because the ldweights lowering pass keeps PSUM
    accumulation groups together and would otherwise reorder the stream.
  - All input DMAs are dependency-free, issued up front on the SP ring in
    compute order (pair0 split kt/vt for an early start, the f16 value
    half last); the ACT ring carries no input DMAs so exp latency stays
    clean. Output blocks leave as soon as their psum->sbuf copy lands,
    alternating SP/ACT rings behind the input issue stream.
"""

import os
import sys

import numpy as np

for _p in ("/opt/trn_rl_repo", "/root/.axon_site/_ro/trn_rl_repo"):
    if os.path.isdir(_p) and _p not in sys.path:
        sys.path.append(_p)

N_CORES = 8
SUB = 128
H = 768
HSUB = H // 128  # 6
CA = 32
VW = 772         # value tile: 768 cols + ones col @768 + pad to 4*193
NQ = VW // 4     # 193
TQW = HSUB * CA  # 192
# fp8 pair block: kt0 | tq | kt1 | vt0 | vt1  (e3m4)
PAIRW = 2 * H + TQW + 2 * VW
KT_OFF = (0, H + TQW)    # kt offset per slot-in-pair
PTQ_OFF = H              # tq offset in pair block
PVT_OFF = 2 * H + TQW    # vt region offset in pair block
KSCALE = 32.0            # fp8 key pre-scale (clears e3m4 subnormal floor)
FP8_MIN_L = int(os.environ.get("BASS_FP8_MIN_L", "300"))

_module_cache = {}


def _build_module(P8, n16):
    """One SPMD module: P8 fp8 pairs (2 slots each) + n16 fp16 slots."""
    import concourse.mybir as mybir
    import concourse.tile as tile
    from concourse import bacc

    f32 = mybir.dt.float32
    f16 = mybir.dt.float16
    f8 = mybir.dt.float8e3
    AF = mybir.ActivationFunctionType

    nout = P8 + n16          # one output block per pair + per f16 slot
    HK16 = n16 * (TQW + H)   # f16 head region: per-slot tq+kt
    a16w = HK16 + n16 * VW   # plus the vt tail region

    nc = bacc.Bacc(None, target_bir_lowering=False, enable_asserts=False)
    a8_d = nc.dram_tensor("a8", [128, max(P8 * PAIRW, 1)], f8, kind="ExternalInput")
    a16_d = nc.dram_tensor("a16", [128, max(a16w, 1)], f16, kind="ExternalInput")
    out_d = nc.dram_tensor("outp", [128, nout * NQ], f16, kind="ExternalOutput")

    with tile.TileContext(nc) as tc:
        with (
            tc.tile_pool(name="stage", bufs=1) as stage,
            tc.tile_pool(name="ps_s", bufs=4, space="PSUM") as ps_s_pool,
            tc.tile_pool(name="al", bufs=8) as al_pool,
            tc.tile_pool(name="ps_o", bufs=3, space="PSUM") as ps_o_pool,
        ):
            a8_t = [
                stage.tile([128, PAIRW], f8, tag=f"a8_{p}", name=f"a8_{p}")
                for p in range(P8)
            ]
            a16_t = (
                stage.tile([128, a16w], f16, tag="a16", name="a16") if n16 else None
            )
            outsb = stage.tile([128, nout * NQ], f16, tag="outsb", name="outsb")

            # input DMAs up front on the SP ring, in compute order
            for p in range(P8):
                if p == 0:
                    s0 = H + TQW  # kt0+tq first so scores start early
                    nc.sync.dma_start(out=a8_t[0][:, :s0], in_=a8_d[:, :s0])
                    nc.sync.dma_start(out=a8_t[0][:, s0:], in_=a8_d[:, s0:PAIRW])
                else:
                    nc.sync.dma_start(
                        out=a8_t[p], in_=a8_d[:, p * PAIRW : (p + 1) * PAIRW]
                    )
            if n16:
                nc.sync.dma_start(out=a16_t[:, :HK16], in_=a16_d[:, :HK16])
                nc.sync.dma_start(out=a16_t[:, HK16:], in_=a16_d[:, HK16:])

            # groups: (list of (kt view, vt view, tq view), exp scale)
            groups = []
            for p in range(P8):
                t = a8_t[p]
                groups.append(
                    (
                        [
                            (
                                t[:, KT_OFF[m] : KT_OFF[m] + H],
                                t[:, PVT_OFF + m * VW : PVT_OFF + (m + 1) * VW],
                                t[:, PTQ_OFF : PTQ_OFF + TQW],
                            )
                            for m in range(2)
                        ],
                        1.0 / KSCALE,
                    )
                )
            for k in range(n16):
                base = k * (TQW + H)
                groups.append(
                    (
                        [
                            (
                                a16_t[:, base + TQW : base + TQW + H],
                                a16_t[:, HK16 + k * VW : HK16 + (k + 1) * VW],
                                a16_t[:, base : base + TQW],
                            )
                        ],
                        1.0,
                    )
                )
            ng = len(groups)

            al_t = {}

            def emit_scores(g):
                members, scale = groups[g]
                w = len(members) * CA
                ps_s = ps_s_pool.tile([128, w], f32, tag="ps_s", name=f"ps_s_{g}")
                als = []
                for gi, (kt_v, _, tq_v) in enumerate(members):
                    for ho in range(HSUB):
                        nc.tensor.matmul(
                            ps_s[:, gi * CA : (gi + 1) * CA],
                            lhsT=kt_v[:, ho * 128 : (ho + 1) * 128],
                            rhs=tq_v[:, ho * CA : (ho + 1) * CA],
                            start=(ho == 0),
                            stop=(ho == HSUB - 1),
                        )
                for gi in range(len(members)):
                    al = al_pool.tile([128, CA], f16, tag="al", name=f"al_{g}_{gi}")
                    nc.scalar.activation(
                        out=al,
                        in_=ps_s[:, gi * CA : (gi + 1) * CA],
                        func=AF.Exp,
                        scale=scale,
                    )
                    als.append(al)
                al_t[g] = als

            def emit_value(g):
                members, _ = groups[g]
                ps_o = ps_o_pool.tile([128, NQ], f32, tag="ps_o", name=f"ps_o_{g}")
                for gi, (_, vt_v, _) in enumerate(members):
                    for j in range(4):
                        nc.tensor.matmul(
                            ps_o[32 * j : 32 * (j + 1), :],
                            lhsT=al_t[g][gi],
                            rhs=vt_v[:, j * NQ : (j + 1) * NQ],
                            start=(gi == 0),
                            stop=(gi == len(members) - 1),
                            tile_position=(0, 32 * j),
                        )
                nc.vector.tensor_copy(out=outsb[:, g * NQ : (g + 1) * NQ], in_=ps_o)
                eng = nc.sync if g % 2 == 0 else nc.scalar
                eng.dma_start(
                    out=out_d[:, g * NQ : (g + 1) * NQ],
                    in_=outsb[:, g * NQ : (g + 1) * NQ],
                )

            # two-group-deep software pipeline on the PE stream
            for g in range(min(3, ng)):
                emit_scores(g)
            for g in range(ng):
                emit_value(g)
                if g + 3 < ng:
                    emit_scores(g + 3)

    nc.compile()
    return nc


def kernel(key, value, query, seq_len, W, b):
    import ml_dtypes

    e3 = ml_dtypes.float8_e3m4
    key = np.ascontiguousarray(np.asarray(key, dtype=np.float32))
    value = np.ascontiguousarray(np.asarray(value, dtype=np.float32))
    query = np.asarray(query, dtype=np.float32)
    W = np.asarray(W, dtype=np.float32)
    bias = np.asarray(b, dtype=np.float32)
    sl = np.asarray(seq_len).astype(np.int64)

    B, S, H_ = key.shape
    assert H_ == H and S % SUB == 0

    # host: tiny projection  tq[b] = tanh(query[b] @ W + bias)  [B, CA, H]
    tq = np.tanh(query.reshape(B * query.shape[1], -1) @ W + bias)
    tq = tq.reshape(B, query.shape[1], H).astype(np.float32)
    # [128, 192] chunk-major transposed layout per batch
    tqT = {
        bi: np.ascontiguousarray(
            tq[bi].T.reshape(HSUB, 128, CA).transpose(1, 0, 2).reshape(128, TQW)
        )
        for bi in range(B)
    }

    # work lists: 128-row sub-chunks of each valid prefix
    subs8, subs16 = [], []  # (batch, s0, nvalid)
    for bi in range(B):
        L = max(1, min(int(sl[bi]), S))
        lst = subs8 if L >= FP8_MIN_L else subs16
        for s0 in range(0, L, SUB):
            lst.append((bi, s0, min(SUB, L - s0)))

    # fp8 subs -> same-batch pairs (a pair shares tq + output accumulator)
    pairs = []
    i = 0
    while i < len(subs8):
        if i + 1 < len(subs8) and subs8[i][0] == subs8[i + 1][0]:
            pairs.append([subs8[i], subs8[i + 1]])
            i += 2
        else:
            pairs.append([subs8[i]])
            i += 1

    P8 = -(-len(pairs) // N_CORES) if pairs else 0
    n16 = -(-len(subs16) // N_CORES) if subs16 else 0
    HK16 = n16 * (TQW + H)

    a8 = np.zeros((N_CORES, 128, max(P8 * PAIRW, 1)), e3)
    a16 = np.zeros((N_CORES, 128, max(HK16 + n16 * VW, 1)), np.float16)
    out_map = [[] for _ in range(N_CORES)]  # per core: (out block, batch)

    def pack_kt(bi, s0, nval, scale):
        kc = key[bi, s0 : s0 + SUB].copy()
        kc[nval:] = 0.0
        return (kc.T * scale).reshape(HSUB, 128, SUB).transpose(1, 0, 2).reshape(128, H)

    def pack_vt(bi, s0, nval):
        vt = np.zeros((128, VW), np.float32)
        vt[:nval, :H] = value[bi, s0 : s0 + nval]
        vt[:nval, H] = 1.0
        return vt

    for pi, pair in enumerate(pairs):
        c, p = pi % N_CORES, pi // N_CORES
        blk = a8[c, :, p * PAIRW : (p + 1) * PAIRW]
        blk[:, PTQ_OFF : PTQ_OFF + TQW] = tqT[pair[0][0]].astype(e3)
        for m, (bi, s0, nval) in enumerate(pair):
            blk[:, KT_OFF[m] : KT_OFF[m] + H] = pack_kt(bi, s0, nval, KSCALE).astype(e3)
            blk[:, PVT_OFF + m * VW : PVT_OFF + (m + 1) * VW] = pack_vt(
                bi, s0, nval
            ).astype(e3)
        out_map[c].append((p, pair[0][0]))

    for si, (bi, s0, nval) in enumerate(subs16):
        c, k = si % N_CORES, si // N_CORES
        base = k * (TQW + H)
        a16[c, :, base : base + TQW] = tqT[bi].astype(np.float16)
        a16[c, :, base + TQW : base + TQW + H] = pack_kt(bi, s0, nval, 1.0)
        a16[c, :, HK16 + k * VW : HK16 + (k + 1) * VW] = pack_vt(bi, s0, nval)
        out_map[c].append((P8 + k, bi))

    cfg = (P8, n16)
    if cfg not in _module_cache:
        _module_cache[cfg] = _build_module(P8, n16)
    nc = _module_cache[cfg]

    from concourse.bass_utils import run_bass_kernel_spmd

    in_maps = [{"a8": a8[c], "a16": a16[c]} for c in range(N_CORES)]
    trace = os.environ.get("BASS_KERNEL_TRACE") == "1"
    kwargs = {}
    if trace:
        kwargs = dict(trace=True, trace_cores=list(range(N_CORES)))
    res = run_bass_kernel_spmd(nc, in_maps, core_ids=list(range(N_CORES)), **kwargs)
    if trace and res.exec_time_ns is not None:
        print(f"HW exec time: {res.exec_time_ns} ns")
        print(f"HW exec time mean: {res.mean_exec_time_ns} ns")

    num = np.zeros((B, CA, H), np.float64)
    den = np.zeros((B, CA), np.float64)
    for c in range(N_CORES):
        part = res.results[c]["outp"]  # [128, nout*NQ]; 4 col-tiled quarters
        for ob, bi in out_map[c]:
            blk = part[:, ob * NQ : (ob + 1) * NQ].astype(np.float64)
            full = np.concatenate(list(blk.reshape(4, CA, NQ)), axis=1)  # [CA, VW]
            num[bi] += full[:, :H]
            den[bi] += full[:, H]
    out = (num / den[:, :, None]).astype(np.float32)
    return out
